# revision 5
# baseline (speedup 1.0000x reference)
"""Trainium2 Bass kernel for GQA multi-head attention with RoPE.

Problem: B=2, T=2048, C=2048, 16 q-heads, 4 kv-heads, HD=128, causal, RoPE.

Sharding (8 cores): tensor-parallel over the 4 kv-head groups x data-parallel
over the 2 batch elements. Core c handles batch c//4, kv-group c%4 (4 q-heads).
Each core computes x @ wq/wk/wv for its head group, RoPE, causal attention,
and a partial output projection (rows of wo for its heads). The host sums the
4 partial outputs per batch element.

v2 design (flipped projection):
- QKV projection computed directly transposed: qT/kT [d, t] via
  matmul(lhsT=wqkv tile, rhs=xT tile) -- no PE transposes for Q/K.
  RoPE pairs live across partitions (perm puts even rope lanes in rows 0:64,
  odd in 64:128); a PSUM->SBUF partition-swap DMA makes both halves
  partition-local for the DVE.
- V is produced as vT [d, s] and turned into v [s, d] with 16 PE transposes.
- Attention: scores [s,t] -> exp (Act, bf16 out) -> softmax denominator via
  ones-matmul, with full exp tiles pre-folded in groups of 4 on the Pool
  engine (bf16 adds) to shrink the PE denominator pass -> AV matmul (bf16)
  -> normalize (DVE). Heads pipelined ~2 deep.
- Output projection row-sharded as before; host sums 4 partials per batch.
"""

import sys

sys.path.insert(0, "/opt/trn_rl_repo")

import numpy as np

B, T, C = 2, 2048, 2048
N_KV = 4
G = 4           # q heads per kv head
HD = 128
NCORES = 8
CT = C // 128   # 16 c-tiles
NTC = 4         # 512-wide t chunks
SCALE = float(1.0 / np.sqrt(HD))
MASK_BIAS = -1.0e5

_CACHE = {}
LAST_RESULTS = None


def _build():
    import concourse.bass as bass
    import concourse.tile as tile
    from concourse import mybir, bacc

    f32 = mybir.dt.float32
    f32r = mybir.dt.float32r
    bf16 = mybir.dt.bfloat16

    nc = bacc.Bacc()
    # [p, ch(4), ct(16), 512]: element = x[b].T[ct*128+p, ch*512+t']
    xT = nc.dram_tensor("xT", [128, 4 * 16 * 512], f32r, kind="ExternalInput")
    # [p, m(6), ct(16), 128]
    wqkv = nc.dram_tensor("wqkv", [128, 6 * 16 * 128], f32r,
                          kind="ExternalInput")
    wo = nc.dram_tensor("wo", [128, G * C], f32r, kind="ExternalInput")
    # duplicated-half rope tables [128, T]: rows 0:64 and 64:128 identical
    fcos2 = nc.dram_tensor("fcos2", [128, T], f32, kind="ExternalInput")
    fsin2 = nc.dram_tensor("fsin2", [128, T], f32, kind="ExternalInput")
    cident = nc.dram_tensor("cident", [128, 128], f32r, kind="ExternalInput")
    cones = nc.dram_tensor("cones", [128, 1], f32, kind="ExternalInput")
    ctri = nc.dram_tensor("ctri", [128, 128], f32, kind="ExternalInput")
    out = nc.dram_tensor("out", [T, C], f32, kind="ExternalOutput")

    with tile.TileContext(nc) as tc:
        with (
            tc.tile_pool(name="consts", bufs=1) as cpool,
            tc.tile_pool(name="persist", bufs=1) as ppool,
        ):
            ident_sb = cpool.tile([128, 128], f32r)
            ones_sb = cpool.tile([128, 1], f32)
            ones_bf = cpool.tile([128, 1], bf16)
            tri_sb = cpool.tile([128, 128], f32)

            # ---- persistent activations ----
            qT_sb = ppool.tile([128, G, T], f32r)       # [d, h, t]
            kT_sb = ppool.tile([128, T], f32r)          # [d, s]
            v_sb = ppool.tile([128, 16, HD], bf16)      # [s%128, s//128, d]
            outT_sb = ppool.tile([128, G, T], f32r)     # [d, h, t]

            # ================= Phase 1: QKV projection + RoPE ================
            with (
                tc.tile_pool(name="weights", bufs=1) as wpool,
                tc.tile_pool(name="freqs", bufs=1) as fpool,
                tc.tile_pool(name="xt", bufs=3) as xtp,
                tc.tile_pool(name="swp", bufs=4) as swp,
                tc.tile_pool(name="ropet", bufs=3) as rtp,
                tc.tile_pool(name="vtmp", bufs=2) as vtp,
                tc.tile_pool(name="pp", bufs=7, space="PSUM") as pp,
                tc.tile_pool(name="pvt", bufs=1, space="PSUM") as pvt,
            ):
                nc.sync.dma_start(ident_sb[:], cident[:])
                wqkv_sb = wpool.tile([128, 6, CT, 128], f32r)
                wflat = wqkv_sb[:].rearrange("p a b c -> p (a b c)")
                for m in range(6):
                    nc.gpsimd.dma_start(
                        wflat[:, m * 2048:(m + 1) * 2048],
                        wqkv[:, m * 2048:(m + 1) * 2048],
                    )
                fcos_sb = fpool.tile([128, T], f32)
                fsin_sb = fpool.tile([128, T], f32)
                nc.gpsimd.dma_start(fcos_sb[:], fcos2[:])
                nc.gpsimd.dma_start(fsin_sb[:], fsin2[:])
                nc.gpsimd.dma_start(ones_sb[:], cones[:])
                nc.gpsimd.dma_start(tri_sb[:], ctri[:])
                nc.vector.tensor_copy(ones_bf[:], ones_sb[:])

                # PE warm-up spin during startup DMAs (pstate ramp ~3.4us)
                warm_ps = pp.tile([128, 512], f32, tag="g", name="warm")
                for _ in range(18):
                    nc.tensor.matmul(warm_ps[:, 0:128], ident_sb[:],
                                     ident_sb[:], start=True, stop=True)

                pending_v = []

                def emit_v_flush():
                    for ch_p, vt_p in pending_v:
                        pv = pvt.tile([128, 4, 128], f32r, tag="pv", name="pv")
                        for i in range(4):
                            nc.tensor.transpose(
                                pv[:, i, :], vt_p[:, i * 128:(i + 1) * 128],
                                ident_sb[:],
                            )
                        for i in range(4):
                            nc.scalar.copy(v_sb[:, ch_p * 4 + i, :],
                                           pv[:, i, :])
                    pending_v.clear()

                for ch in range(NTC):
                    scope = nc.named_scope(f"p1_ch{ch}")
                    scope.__enter__()
                    groups = [pp.tile([128, 512], f32, tag="g",
                                      name=f"g{m}") for m in range(6)]
                    for qt in range(4):
                        xt = xtp.tile([128, 4, 512], f32r, tag="xt")
                        col0 = ch * 8192 + qt * 2048
                        nc.sync.dma_start(
                            xt[:].rearrange("p a b -> p (a b)"),
                            xT[:, col0:col0 + 2048],
                        )
                        for m in range(6):
                            for ci in range(4):
                                ct = qt * 4 + ci
                                nc.tensor.matmul(
                                    groups[m][:], wqkv_sb[:, m, ct, :],
                                    xt[:, ci, :],
                                    start=(ct == 0), stop=(ct == CT - 1),
                                )
                    # transposes for the PREVIOUS chunk's V (keeps PE dense)
                    emit_v_flush()
                    # rope q heads + k: psum->sbuf copy, partition-swap DMA
                    # (sbuf->sbuf), then partition-local DVE ops
                    tcols = slice(ch * 512, (ch + 1) * 512)
                    for m in range(5):
                        ps = groups[m]
                        cp = swp.tile([128, 512], f32, tag="cp")
                        nc.scalar.copy(cp[:], ps[:])
                        sw = swp.tile([128, 512], f32, tag="sw")
                        nc.gpsimd.dma_start(sw[0:64, :], cp[64:128, :])
                        nc.gpsimd.dma_start(sw[64:128, :], cp[0:64, :])
                        if m < 4:
                            dhi = qT_sb[0:64, m, tcols]
                            dlo = qT_sb[64:128, m, tcols]
                        else:
                            dhi = kT_sb[0:64, tcols]
                            dlo = kT_sb[64:128, tcols]
                        t1 = rtp.tile([128, 512], f32, tag="rt")
                        t2 = rtp.tile([128, 512], f32, tag="rt")
                        # re = te*cos - to*sin  (partitions 0:64)
                        nc.vector.tensor_mul(t1[0:64, :], cp[0:64, :],
                                             fcos_sb[0:64, tcols])
                        nc.vector.tensor_mul(t2[0:64, :], sw[0:64, :],
                                             fsin_sb[0:64, tcols])
                        nc.vector.tensor_sub(dhi, t1[0:64, :], t2[0:64, :])
                        # im = te*sin + to*cos  (partitions 64:128)
                        nc.vector.tensor_mul(t1[64:128, :], sw[64:128, :],
                                             fsin_sb[64:128, tcols])
                        nc.vector.tensor_mul(t2[64:128, :], cp[64:128, :],
                                             fcos_sb[64:128, tcols])
                        nc.vector.tensor_add(dlo, t1[64:128, :],
                                             t2[64:128, :])
                    # v: copy psum -> sbuf now; transpose during next chunk
                    vt = vtp.tile([128, 512], f32r, tag="vt")
                    nc.scalar.copy(vt[:], groups[5][:])
                    pending_v.append((ch, vt))
                    scope.__exit__(None, None, None)
                emit_v_flush()

            # ================= Phase 2+3: attention + output projection ======
            with (
                tc.tile_pool(name="wop", bufs=1) as wop,
                tc.tile_pool(name="expt", bufs=3) as expp,
                tc.tile_pool(name="esum", bufs=2) as esp,
                tc.tile_pool(name="ftmp", bufs=4) as ftp,
                tc.tile_pool(name="denb", bufs=2) as denp,
                tc.tile_pool(name="bcb", bufs=2) as bcp,
                tc.tile_pool(name="outbuf", bufs=4) as opool,
                tc.tile_pool(name="pssc", bufs=3, space="PSUM") as pssc,
                tc.tile_pool(name="psden", bufs=1, space="PSUM") as psden,
                tc.tile_pool(name="psav", bufs=2, space="PSUM") as psav,
                tc.tile_pool(name="pswo", bufs=2, space="PSUM") as pswo,
            ):
                wo_sb = wop.tile([128, G, C], f32r)
                nc.gpsimd.dma_start(wo_sb[:].rearrange("p a b -> p (a b)"),
                                    wo[:])

                def emit_scores(tc_i, h):
                    t0 = tc_i * 512
                    n_s = 4 * (tc_i + 1)
                    expT = expp.tile([128, 16, 512], bf16, tag="expT",
                                     name="expT")
                    for si in range(n_s):
                        off = 128 * (si - 4 * tc_i) if si >= 4 * tc_i else 0
                        ps = pssc.tile([128, 512], f32, tag="sc", name="ps")
                        nc.tensor.matmul(
                            ps[:, off:512],
                            kT_sb[:, si * 128:(si + 1) * 128],
                            qT_sb[:, h, t0 + off:t0 + 512],
                            start=True, stop=True,
                        )
                        if si >= 4 * tc_i:
                            nc.vector.tensor_add(
                                ps[:, off:off + 128],
                                ps[:, off:off + 128], tri_sb[:],
                            )
                        nc.scalar.activation(
                            expT[:, si, off:512], ps[:, off:512],
                            mybir.ActivationFunctionType.Exp, scale=SCALE,
                        )
                    return expT

                def emit_fold(tc_i, h, expT):
                    # fold full (non-diagonal) exp tiles in groups of 4 on
                    # the Pool engine; shrinks the PE denominator pass
                    es = esp.tile([128, 3, 512], bf16, tag="es", name="es")
                    for g in range(tc_i):
                        a1 = ftp.tile([128, 512], bf16, tag="ft")
                        a2 = ftp.tile([128, 512], bf16, tag="ft")
                        nc.gpsimd.tensor_add(a1[:], expT[:, 4 * g, :],
                                             expT[:, 4 * g + 1, :])
                        nc.gpsimd.tensor_add(a2[:], expT[:, 4 * g + 2, :],
                                             expT[:, 4 * g + 3, :])
                        nc.gpsimd.tensor_add(es[:, g, :], a1[:], a2[:])
                    return es

                def emit_den(tc_i, h, expT, es):
                    # denominator: diag-first (full 512) then folded groups
                    # then trimmed diagonal tiles
                    seq = [(expT[:, 4 * tc_i, 0:512], 0)]
                    seq += [(es[:, g, :], 0) for g in range(tc_i)]
                    for j in range(1, 4):
                        off = 128 * j
                        seq.append((expT[:, 4 * tc_i + j, off:512], off))
                    psd = psden.tile([1, 512], f32, tag="den", name="psd")
                    for i, (rhs, off) in enumerate(seq):
                        nc.tensor.matmul(psd[:, off:512], ones_bf[:], rhs,
                                         start=(i == 0),
                                         stop=(i == len(seq) - 1))
                    den_r = denp.tile([1, 512], f32, tag="denr", name="dr")
                    nc.vector.reciprocal_approx_fast(den_r[:], psd[:])
                    bc = bcp.tile([128, 512], f32, tag="bc", name="bc")
                    nc.gpsimd.partition_broadcast(bc[:], den_r[:])
                    return bc

                def emit_av(tc_i, h, expT, bc):
                    t0 = tc_i * 512
                    n_s = 4 * (tc_i + 1)
                    order = [4 * tc_i] + [si for si in range(n_s)
                                          if si != 4 * tc_i]
                    pso = psav.tile([128, 512], f32, tag="av", name="pso")
                    for i, si in enumerate(order):
                        off = 128 * (si - 4 * tc_i) if si >= 4 * tc_i else 0
                        nc.tensor.matmul(
                            pso[:, off:512], v_sb[:, si, :],
                            expT[:, si, off:512],
                            start=(i == 0), stop=(i == n_s - 1),
                        )
                    nc.vector.tensor_mul(
                        outT_sb[:, h, t0:t0 + 512], pso[:], bc[:]
                    )

                def emit_attn(tc_i):
                    scope = nc.named_scope(f"attn_tc{tc_i}")
                    scope.__enter__()
                    # pipeline: s0 s1 f0 d0 s2 f1 av0 d1 s3 f2 av1 d2 f3 d3
                    #           av2 av3
                    e = {}
                    es = {}
                    bc = {}
                    e[0] = emit_scores(tc_i, 0)
                    e[1] = emit_scores(tc_i, 1)
                    es[0] = emit_fold(tc_i, 0, e[0])
                    bc[0] = emit_den(tc_i, 0, e[0], es[0])
                    e[2] = emit_scores(tc_i, 2)
                    es[1] = emit_fold(tc_i, 1, e[1])
                    emit_av(tc_i, 0, e[0], bc[0])
                    bc[1] = emit_den(tc_i, 1, e[1], es[1])
                    e[3] = emit_scores(tc_i, 3)
                    es[2] = emit_fold(tc_i, 2, e[2])
                    emit_av(tc_i, 1, e[1], bc[1])
                    bc[2] = emit_den(tc_i, 2, e[2], es[2])
                    es[3] = emit_fold(tc_i, 3, e[3])
                    bc[3] = emit_den(tc_i, 3, e[3], es[3])
                    emit_av(tc_i, 2, e[2], bc[2])
                    emit_av(tc_i, 3, e[3], bc[3])
                    scope.__exit__(None, None, None)

                def emit_wo(tc_i):
                    scope = nc.named_scope(f"wo_tc{tc_i}")
                    scope.__enter__()
                    for t2 in range(4):
                        gt = tc_i * 4 + t2
                        for cc in range(4):
                            psw = pswo.tile([128, 512], f32, tag="wo",
                                            name="psw")
                            for h in range(G):
                                nc.tensor.matmul(
                                    psw[:],
                                    outT_sb[:, h, gt * 128:(gt + 1) * 128],
                                    wo_sb[:, h, cc * 512:(cc + 1) * 512],
                                    start=(h == 0), stop=(h == G - 1),
                                )
                            osb = opool.tile([128, 512], f32, tag="osb",
                                             name="osb")
                            if (t2 * 4 + cc) % 2:
                                nc.vector.tensor_copy(osb[:], psw[:])
                            else:
                                nc.scalar.copy(osb[:], psw[:])
                            nc.sync.dma_start(
                                out[gt * 128:(gt + 1) * 128,
                                    cc * 512:(cc + 1) * 512],
                                osb[:],
                            )
                    scope.__exit__(None, None, None)

                # wo load races phase-1's SBUF release; run attn tc0+tc1
                # before the first wo block to hide it
                emit_attn(0)
                emit_attn(1)
                emit_wo(0)
                emit_attn(2)
                emit_wo(1)
                emit_attn(3)
                emit_wo(2)
                emit_wo(3)

    nc.finalize()
    return nc


def _prep_host(x, freqs_cos, freqs_sin, wq, wk, wv, wo):
    """Build per-core input maps."""
    x = np.asarray(x, dtype=np.float32)
    freqs_cos = np.asarray(freqs_cos, dtype=np.float32)
    freqs_sin = np.asarray(freqs_sin, dtype=np.float32)
    wq = np.asarray(wq, dtype=np.float32)
    wk = np.asarray(wk, dtype=np.float32)
    wv = np.asarray(wv, dtype=np.float32)
    wo = np.asarray(wo, dtype=np.float32)

    perm = np.concatenate([np.arange(0, HD, 2), np.arange(1, HD, 2)])
    # xT pre-tiled: [p, ch, ct, t'] so each (ch, qt) load is contiguous
    xTs = []
    for b in range(B):
        A = np.ascontiguousarray(x[b].T)           # [C, T]
        A = A.reshape(CT, 128, 4, 512)             # [ct, p, ch, t']
        A = A.transpose(1, 2, 0, 3)                # [p, ch, ct, t']
        xTs.append(np.ascontiguousarray(A.reshape(128, -1)))
    cident = np.eye(128, dtype=np.float32)
    cones = np.ones((128, 1), dtype=np.float32)
    ds, dt = np.meshgrid(np.arange(128), np.arange(128), indexing="ij")
    ctri = np.where(ds <= dt, 0.0, MASK_BIAS).astype(np.float32)
    fcos2 = np.ascontiguousarray(
        np.concatenate([freqs_cos.T, freqs_cos.T], axis=0))  # [128, T]
    fsin2 = np.ascontiguousarray(
        np.concatenate([freqs_sin.T, freqs_sin.T], axis=0))

    in_maps = []
    for c in range(NCORES):
        b, kv = c // 4, c % 4
        cols = []
        for g in range(G):
            h = kv * G + g
            cols.append(wq[:, h * HD:(h + 1) * HD][:, perm])
        cols.append(wk[:, kv * HD:(kv + 1) * HD][:, perm])
        cols.append(wv[:, kv * HD:(kv + 1) * HD])
        wqkv_c = np.concatenate(cols, axis=1)              # [C, 768]
        # [C, 6*128] -> [p, m, ct, d]
        wqkv_c = wqkv_c.reshape(CT, 128, 6, 128).transpose(1, 2, 0, 3)
        wqkv_c = np.ascontiguousarray(wqkv_c.reshape(128, -1))
        wo_c = wo[kv * G * HD:(kv + 1) * G * HD, :]        # [512, C]
        wo_c = wo_c.reshape(G, 128, C).transpose(1, 0, 2)
        wo_c = np.ascontiguousarray(wo_c.reshape(128, -1))
        in_maps.append({
            "xT": xTs[b],
            "wqkv": wqkv_c,
            "wo": wo_c,
            "fcos2": fcos2,
            "fsin2": fsin2,
            "cident": cident,
            "cones": cones,
            "ctri": ctri,
        })
    return in_maps


def _install_ntff_hook_shim():
    """bass_utils trace=True needs antenv.axon_hooks, absent in this image.
    Provide it in sys.modules and register the ctypes NTFF hook."""
    import types

    if "antenv.axon_hooks" in sys.modules:
        return
    mod = types.ModuleType("antenv.axon_hooks")
    mod._hook = None
    mod.set_axon_ntff_profile_hook = lambda h: setattr(mod, "_hook", h)
    mod.get_axon_ntff_profile_hook = lambda: mod._hook
    sys.modules["antenv.axon_hooks"] = mod
    try:
        from trn_agent_boot.trn_boot import _ntff_profile_via_ctypes

        mod._hook = _ntff_profile_via_ctypes("/opt/axon/libaxon_pjrt.so")
    except Exception:
        pass


def kernel(x, freqs_cos, freqs_sin, wq, wk, wv, wo, trace=False):
    global LAST_RESULTS
    from concourse.bass_utils import run_bass_kernel_spmd

    if trace:
        _install_ntff_hook_shim()

    if "nc" not in _CACHE:
        _CACHE["nc"] = _build()
    nc = _CACHE["nc"]

    in_maps = _prep_host(x, freqs_cos, freqs_sin, wq, wk, wv, wo)
    res = run_bass_kernel_spmd(nc, in_maps, core_ids=list(range(NCORES)),
                               trace=trace)
    LAST_RESULTS = res
    out = np.zeros((B, T, C), dtype=np.float32)
    for c in range(NCORES):
        out[c // 4] += res.results[c]["out"]
    return out


# revision 11
# speedup vs baseline: 1.5776x; 1.5776x over previous
"""Trainium2 Bass kernel for GQA multi-head attention with RoPE.

Problem: B=2, T=2048, C=2048, 16 q-heads, 4 kv-heads, HD=128, causal, RoPE.

Sharding (8 cores): tensor-parallel over the 4 kv-head groups x data-parallel
over the 2 batch elements. Core c handles batch c//4, kv-group c%4 (4 q-heads).
Each core computes x @ wq/wk/wv for its head group, RoPE, causal attention,
and a partial output projection (rows of wo for its heads). The host sums the
4 partial outputs per batch element.

v2 design (flipped projection):
- QKV projection computed directly transposed: qT/kT [d, t] via
  matmul(lhsT=wqkv tile, rhs=xT tile) -- no PE transposes for Q/K.
  RoPE pairs live across partitions (perm puts even rope lanes in rows 0:64,
  odd in 64:128); a PSUM->SBUF partition-swap DMA makes both halves
  partition-local for the DVE.
- V is produced as vT [d, s] and turned into v [s, d] with 16 PE transposes.
- Attention: scores [s,t] -> exp (Act, bf16 out) -> softmax denominator via
  ones-matmul, with full exp tiles pre-folded in groups of 4 on the Pool
  engine (bf16 adds) to shrink the PE denominator pass -> AV matmul (bf16)
  -> normalize (DVE). Heads pipelined ~2 deep.
- Output projection row-sharded as before; host sums 4 partials per batch.
"""

import sys

sys.path.insert(0, "/opt/trn_rl_repo")

import numpy as np

B, T, C = 2, 2048, 2048
N_KV = 4
G = 4           # q heads per kv head
HD = 128
NCORES = 8
CT = C // 128   # 16 c-tiles
NTC = 4         # 512-wide t chunks
SCALE = float(1.0 / np.sqrt(HD))
MASK_BIAS = -1.0e5

_CACHE = {}
LAST_RESULTS = None


def _build():
    import concourse.bass as bass
    import concourse.tile as tile
    from concourse import mybir, bacc

    f32 = mybir.dt.float32
    f32r = mybir.dt.float32r
    bf16 = mybir.dt.bfloat16

    nc = bacc.Bacc()
    # [p, ch(4), ct(16), 512]: element = x[b].T[ct*128+p, ch*512+t']
    xT = nc.dram_tensor("xT", [128, 4 * 16 * 512], f32r, kind="ExternalInput")
    # [p, m(6), ct(16), 128]
    wqkv = nc.dram_tensor("wqkv", [128, 6 * 16 * 128], f32r,
                          kind="ExternalInput")
    wo = nc.dram_tensor("wo", [128, G * C], f32r, kind="ExternalInput")
    # duplicated-half rope tables [128, T]: rows 0:64 and 64:128 identical
    fcos2 = nc.dram_tensor("fcos2", [128, T], f32, kind="ExternalInput")
    fsin2 = nc.dram_tensor("fsin2", [128, T], f32, kind="ExternalInput")
    cident = nc.dram_tensor("cident", [128, 128], f32r, kind="ExternalInput")
    cones = nc.dram_tensor("cones", [128, 1], f32, kind="ExternalInput")
    ctri = nc.dram_tensor("ctri", [128, 128], f32, kind="ExternalInput")
    out = nc.dram_tensor("out", [T, C], f32, kind="ExternalOutput")

    with tile.TileContext(nc) as tc:
        with (
            tc.tile_pool(name="consts", bufs=1) as cpool,
            tc.tile_pool(name="persist", bufs=1) as ppool,
        ):
            ident_sb = cpool.tile([128, 128], f32r)
            ones_sb = cpool.tile([128, 1], f32)
            ones_bf = cpool.tile([128, 1], bf16)
            tri_sb = cpool.tile([128, 128], f32)

            # ---- persistent activations ----
            qT_sb = ppool.tile([128, G, T], f32r)       # [d, h, t]
            kT_sb = ppool.tile([128, T], f32r)          # [d, s]
            v_sb = ppool.tile([128, 16, HD], bf16)      # [s%128, s//128, d]
            outT_sb = ppool.tile([128, G, T], f32r)     # [d, h, t]

            # ================= Phase 1: QKV projection + RoPE ================
            with (
                tc.tile_pool(name="weights", bufs=1) as wpool,
                tc.tile_pool(name="freqs", bufs=1) as fpool,
                tc.tile_pool(name="xt", bufs=3) as xtp,
                tc.tile_pool(name="swp", bufs=4) as swp,
                tc.tile_pool(name="ropet", bufs=3) as rtp,
                tc.tile_pool(name="vtmp", bufs=2) as vtp,
                tc.tile_pool(name="pp", bufs=7, space="PSUM") as pp,
                tc.tile_pool(name="pvt", bufs=1, space="PSUM") as pvt,
            ):
                nc.sync.dma_start(ident_sb[:], cident[:])
                # ct-major so load order matches qt-major consumption
                wqkv_sb = wpool.tile([128, CT, 6, 128], f32r)
                wflat = wqkv_sb[:].rearrange("p a b c -> p (a b c)")
                for ct in range(CT):
                    nc.gpsimd.dma_start(
                        wflat[:, ct * 768:(ct + 1) * 768],
                        wqkv[:, ct * 768:(ct + 1) * 768],
                    )
                fcos_sb = fpool.tile([128, T], f32)
                fsin_sb = fpool.tile([128, T], f32)
                nc.gpsimd.dma_start(fcos_sb[:], fcos2[:])
                nc.gpsimd.dma_start(fsin_sb[:], fsin2[:])
                nc.gpsimd.dma_start(ones_sb[:], cones[:])
                nc.gpsimd.dma_start(tri_sb[:], ctri[:])
                nc.vector.tensor_copy(ones_bf[:], ones_sb[:])

                # PE warm-up spin during startup DMAs (pstate ramp ~3.4us)
                warm_ps = pp.tile([128, 512], f32, tag="g", name="warm")
                for _ in range(18):
                    nc.tensor.matmul(warm_ps[:, 0:128], ident_sb[:],
                                     ident_sb[:], start=True, stop=True)

                pending_v = []

                def emit_v_flush():
                    for ch_p, vt_p in pending_v:
                        pv = pvt.tile([128, 4, 128], f32r, tag="pv", name="pv")
                        for i in range(4):
                            nc.tensor.transpose(
                                pv[:, i, :], vt_p[:, i * 128:(i + 1) * 128],
                                ident_sb[:],
                            )
                        for i in range(4):
                            nc.scalar.copy(v_sb[:, ch_p * 4 + i, :],
                                           pv[:, i, :])
                    pending_v.clear()

                for ch in range(NTC):
                    scope = nc.named_scope(f"p1_ch{ch}")
                    scope.__enter__()
                    groups = [pp.tile([128, 512], f32, tag="g",
                                      name=f"g{m}") for m in range(6)]
                    for qt in range(4):
                        xt = xtp.tile([128, 4, 512], f32r, tag="xt")
                        col0 = ch * 8192 + qt * 2048
                        nc.sync.dma_start(
                            xt[:].rearrange("p a b -> p (a b)"),
                            xT[:, col0:col0 + 2048],
                        )
                        for m in range(6):
                            for ci in range(4):
                                ct = qt * 4 + ci
                                nc.tensor.matmul(
                                    groups[m][:], wqkv_sb[:, ct, m, :],
                                    xt[:, ci, :],
                                    start=(ct == 0), stop=(ct == CT - 1),
                                )
                    # transposes for the PREVIOUS chunk's V (keeps PE dense)
                    emit_v_flush()
                    # rope q heads + k: psum->sbuf copy, partition-swap DMA
                    # (sbuf->sbuf), then partition-local DVE ops
                    tcols = slice(ch * 512, (ch + 1) * 512)
                    for m in range(5):
                        ps = groups[m]
                        cp = swp.tile([128, 512], f32, tag="cp")
                        nc.scalar.copy(cp[:], ps[:])
                        sw = swp.tile([128, 512], f32, tag="sw")
                        nc.gpsimd.dma_start(sw[0:64, :], cp[64:128, :])
                        nc.gpsimd.dma_start(sw[64:128, :], cp[0:64, :])
                        dst = (qT_sb[:, m, tcols] if m < 4
                               else kT_sb[:, tcols])
                        t1 = rtp.tile([128, 512], f32, tag="rt")
                        t2 = rtp.tile([128, 512], f32, tag="rt")
                        # cp = [te; to], sw = [to; te]; fcos = [cos; cos],
                        # fsin = [-sin; +sin] (sign baked in on host), so
                        # cp*fcos + sw*fsin = [te*cos - to*sin;
                        #                      to*cos + te*sin] = [re; im]
                        nc.vector.tensor_mul(t1[:], cp[:],
                                             fcos_sb[:, tcols])
                        nc.vector.tensor_mul(t2[:], sw[:],
                                             fsin_sb[:, tcols])
                        nc.vector.tensor_add(dst, t1[:], t2[:])
                    # v: copy psum -> sbuf now; transpose during next chunk
                    vt = vtp.tile([128, 512], f32r, tag="vt")
                    nc.scalar.copy(vt[:], groups[5][:])
                    pending_v.append((ch, vt))
                    scope.__exit__(None, None, None)
                emit_v_flush()

            # ================= Phase 2+3: attention + output projection ======
            with (
                tc.tile_pool(name="wop", bufs=1) as wop,
                tc.tile_pool(name="expt", bufs=3) as expp,
                tc.tile_pool(name="esum", bufs=2) as esp,
                tc.tile_pool(name="ftmp", bufs=4) as ftp,
                tc.tile_pool(name="denb", bufs=2) as denp,
                tc.tile_pool(name="bcb", bufs=2) as bcp,
                tc.tile_pool(name="outbuf", bufs=4) as opool,
                tc.tile_pool(name="pssc", bufs=3, space="PSUM") as pssc,
                tc.tile_pool(name="psden", bufs=1, space="PSUM") as psden,
                tc.tile_pool(name="psav", bufs=2, space="PSUM") as psav,
                tc.tile_pool(name="pswo", bufs=2, space="PSUM") as pswo,
            ):
                wo_sb = wop.tile([128, G, C], f32r)
                nc.gpsimd.dma_start(wo_sb[:].rearrange("p a b -> p (a b)"),
                                    wo[:])

                def emit_scores(tc_i, h):
                    t0 = tc_i * 512
                    n_s = 4 * (tc_i + 1)
                    expT = expp.tile([128, 16, 512], bf16, tag="expT",
                                     name="expT")
                    for si in range(n_s):
                        off = 128 * (si - 4 * tc_i) if si >= 4 * tc_i else 0
                        ps = pssc.tile([128, 512], f32, tag="sc", name="ps")
                        nc.tensor.matmul(
                            ps[:, off:512],
                            kT_sb[:, si * 128:(si + 1) * 128],
                            qT_sb[:, h, t0 + off:t0 + 512],
                            start=True, stop=True,
                        )
                        if si >= 4 * tc_i:
                            nc.vector.tensor_add(
                                ps[:, off:off + 128],
                                ps[:, off:off + 128], tri_sb[:],
                            )
                        nc.scalar.activation(
                            expT[:, si, off:512], ps[:, off:512],
                            mybir.ActivationFunctionType.Exp, scale=SCALE,
                        )
                    return expT

                def emit_fold(tc_i, h, expT):
                    # fold full (non-diagonal) exp tiles in groups of 4 on
                    # the Pool engine; shrinks the PE denominator pass
                    es = esp.tile([128, 3, 512], bf16, tag="es", name="es")
                    for g in range(tc_i):
                        a1 = ftp.tile([128, 512], bf16, tag="ft")
                        a2 = ftp.tile([128, 512], bf16, tag="ft")
                        nc.vector.tensor_add(a1[:], expT[:, 4 * g, :],
                                             expT[:, 4 * g + 1, :])
                        nc.vector.tensor_add(a2[:], expT[:, 4 * g + 2, :],
                                             expT[:, 4 * g + 3, :])
                        nc.vector.tensor_add(es[:, g, :], a1[:], a2[:])
                    return es

                def emit_den(tc_i, h, expT, es):
                    # denominator: diag-first (full 512) then folded groups
                    # then trimmed diagonal tiles
                    seq = [(expT[:, 4 * tc_i, 0:512], 0)]
                    seq += [(es[:, g, :], 0) for g in range(tc_i)]
                    for j in range(1, 4):
                        off = 128 * j
                        seq.append((expT[:, 4 * tc_i + j, off:512], off))
                    psd = psden.tile([1, 512], f32, tag="den", name="psd")
                    for i, (rhs, off) in enumerate(seq):
                        nc.tensor.matmul(psd[:, off:512], ones_bf[:], rhs,
                                         start=(i == 0),
                                         stop=(i == len(seq) - 1))
                    den_r = denp.tile([1, 512], f32, tag="denr", name="dr")
                    nc.vector.reciprocal_approx_fast(den_r[:], psd[:])
                    bc = bcp.tile([128, 512], f32, tag="bc", name="bc")
                    nc.gpsimd.partition_broadcast(bc[:], den_r[:])
                    return bc

                def emit_av(tc_i, h, expT, bc):
                    t0 = tc_i * 512
                    n_s = 4 * (tc_i + 1)
                    order = [4 * tc_i] + [si for si in range(n_s)
                                          if si != 4 * tc_i]
                    pso = psav.tile([128, 512], f32, tag="av", name="pso")
                    for i, si in enumerate(order):
                        off = 128 * (si - 4 * tc_i) if si >= 4 * tc_i else 0
                        nc.tensor.matmul(
                            pso[:, off:512], v_sb[:, si, :],
                            expT[:, si, off:512],
                            start=(i == 0), stop=(i == n_s - 1),
                        )
                    nc.vector.tensor_mul(
                        outT_sb[:, h, t0:t0 + 512], pso[:], bc[:]
                    )

                def emit_attn(tc_i):
                    scope = nc.named_scope(f"attn_tc{tc_i}")
                    scope.__enter__()
                    # pipeline: s0 s1 f0 d0 s2 f1 av0 d1 s3 f2 av1 d2 f3 d3
                    #           av2 av3
                    e = {}
                    es = {}
                    bc = {}
                    e[0] = emit_scores(tc_i, 0)
                    e[1] = emit_scores(tc_i, 1)
                    es[0] = emit_fold(tc_i, 0, e[0])
                    bc[0] = emit_den(tc_i, 0, e[0], es[0])
                    e[2] = emit_scores(tc_i, 2)
                    es[1] = emit_fold(tc_i, 1, e[1])
                    emit_av(tc_i, 0, e[0], bc[0])
                    bc[1] = emit_den(tc_i, 1, e[1], es[1])
                    e[3] = emit_scores(tc_i, 3)
                    es[2] = emit_fold(tc_i, 2, e[2])
                    emit_av(tc_i, 1, e[1], bc[1])
                    bc[2] = emit_den(tc_i, 2, e[2], es[2])
                    es[3] = emit_fold(tc_i, 3, e[3])
                    bc[3] = emit_den(tc_i, 3, e[3], es[3])
                    emit_av(tc_i, 2, e[2], bc[2])
                    emit_av(tc_i, 3, e[3], bc[3])
                    scope.__exit__(None, None, None)

                def emit_wo(tc_i):
                    scope = nc.named_scope(f"wo_tc{tc_i}")
                    scope.__enter__()
                    for t2 in range(4):
                        gt = tc_i * 4 + t2
                        for cc in range(4):
                            psw = pswo.tile([128, 512], f32, tag="wo",
                                            name="psw")
                            for h in range(G):
                                nc.tensor.matmul(
                                    psw[:],
                                    outT_sb[:, h, gt * 128:(gt + 1) * 128],
                                    wo_sb[:, h, cc * 512:(cc + 1) * 512],
                                    start=(h == 0), stop=(h == G - 1),
                                )
                            osb = opool.tile([128, 512], f32, tag="osb",
                                             name="osb")
                            if (t2 * 4 + cc) % 2:
                                nc.vector.tensor_copy(osb[:], psw[:])
                            else:
                                nc.scalar.copy(osb[:], psw[:])
                            nc.sync.dma_start(
                                out[gt * 128:(gt + 1) * 128,
                                    cc * 512:(cc + 1) * 512],
                                osb[:],
                            )
                    scope.__exit__(None, None, None)

                # wo load races phase-1's SBUF release; run attn tc0+tc1
                # before the first wo block to hide it
                emit_attn(0)
                emit_attn(1)
                emit_wo(0)
                emit_attn(2)
                emit_wo(1)
                emit_attn(3)
                emit_wo(2)
                emit_wo(3)

    nc.finalize()
    return nc


def _prep_host(x, freqs_cos, freqs_sin, wq, wk, wv, wo):
    """Build per-core input maps."""
    x = np.asarray(x, dtype=np.float32)
    freqs_cos = np.asarray(freqs_cos, dtype=np.float32)
    freqs_sin = np.asarray(freqs_sin, dtype=np.float32)
    wq = np.asarray(wq, dtype=np.float32)
    wk = np.asarray(wk, dtype=np.float32)
    wv = np.asarray(wv, dtype=np.float32)
    wo = np.asarray(wo, dtype=np.float32)

    perm = np.concatenate([np.arange(0, HD, 2), np.arange(1, HD, 2)])
    # xT pre-tiled: [p, ch, ct, t'] so each (ch, qt) load is contiguous
    xTs = []
    for b in range(B):
        A = np.ascontiguousarray(x[b].T)           # [C, T]
        A = A.reshape(CT, 128, 4, 512)             # [ct, p, ch, t']
        A = A.transpose(1, 2, 0, 3)                # [p, ch, ct, t']
        xTs.append(np.ascontiguousarray(A.reshape(128, -1)))
    cident = np.eye(128, dtype=np.float32)
    cones = np.ones((128, 1), dtype=np.float32)
    ds, dt = np.meshgrid(np.arange(128), np.arange(128), indexing="ij")
    ctri = np.where(ds <= dt, 0.0, MASK_BIAS).astype(np.float32)
    fcos2 = np.ascontiguousarray(
        np.concatenate([freqs_cos.T, freqs_cos.T], axis=0))  # [128, T]
    fsin2 = np.ascontiguousarray(
        np.concatenate([-freqs_sin.T, freqs_sin.T], axis=0))  # sign baked in

    in_maps = []
    for c in range(NCORES):
        b, kv = c // 4, c % 4
        cols = []
        for g in range(G):
            h = kv * G + g
            cols.append(wq[:, h * HD:(h + 1) * HD][:, perm])
        cols.append(wk[:, kv * HD:(kv + 1) * HD][:, perm])
        cols.append(wv[:, kv * HD:(kv + 1) * HD])
        wqkv_c = np.concatenate(cols, axis=1)              # [C, 768]
        # [C, 6*128] -> [p, ct, m, d]
        wqkv_c = wqkv_c.reshape(CT, 128, 6, 128).transpose(1, 0, 2, 3)
        wqkv_c = np.ascontiguousarray(wqkv_c.reshape(128, -1))
        wo_c = wo[kv * G * HD:(kv + 1) * G * HD, :]        # [512, C]
        wo_c = wo_c.reshape(G, 128, C).transpose(1, 0, 2)
        wo_c = np.ascontiguousarray(wo_c.reshape(128, -1))
        in_maps.append({
            "xT": xTs[b],
            "wqkv": wqkv_c,
            "wo": wo_c,
            "fcos2": fcos2,
            "fsin2": fsin2,
            "cident": cident,
            "cones": cones,
            "ctri": ctri,
        })
    return in_maps


def _install_ntff_hook_shim():
    """bass_utils trace=True needs antenv.axon_hooks, absent in this image.
    Provide it in sys.modules and register the ctypes NTFF hook."""
    import types

    if "antenv.axon_hooks" in sys.modules:
        return
    mod = types.ModuleType("antenv.axon_hooks")
    mod._hook = None
    mod.set_axon_ntff_profile_hook = lambda h: setattr(mod, "_hook", h)
    mod.get_axon_ntff_profile_hook = lambda: mod._hook
    sys.modules["antenv.axon_hooks"] = mod
    try:
        from trn_agent_boot.trn_boot import _ntff_profile_via_ctypes

        mod._hook = _ntff_profile_via_ctypes("/opt/axon/libaxon_pjrt.so")
    except Exception:
        pass


def kernel(x, freqs_cos, freqs_sin, wq, wk, wv, wo, trace=False):
    global LAST_RESULTS
    from concourse.bass_utils import run_bass_kernel_spmd

    if trace:
        _install_ntff_hook_shim()

    if "nc" not in _CACHE:
        _CACHE["nc"] = _build()
    nc = _CACHE["nc"]

    in_maps = _prep_host(x, freqs_cos, freqs_sin, wq, wk, wv, wo)
    res = run_bass_kernel_spmd(nc, in_maps, core_ids=list(range(NCORES)),
                               trace=trace)
    LAST_RESULTS = res
    out = np.zeros((B, T, C), dtype=np.float32)
    for c in range(NCORES):
        out[c // 4] += res.results[c]["out"]
    return out
